# revision 16
# baseline (speedup 1.0000x reference)
"""MoE layer (dense routing, E=8 experts, top_k=E) Trainium2 Bass kernel.

Problem (hardcoded): x [4, 2048, 1024] f32, Wg [1024, 8], bg [8],
W1 [8, 1024, 256], b1 [8, 256], W2 [8, 256, 1024], b2 [8, 1024].

reference:
    logits = x @ Wg + bg ; probs = softmax(logits)
    sorted_probs = sort(probs, descending)          # top_k with k=E
    h_e = gelu(x @ W1[e] + b1[e])                   # all experts, all tokens
    out = sum_e (h_e @ W2[e] + b2[e]) * sorted_probs[..., e, None]

Sharding: data-parallel over the 8192 tokens -> 1024 tokens/core, 8 cores,
weights replicated, no collectives.

Per-core layout: activations are feature-major ([D, tok] / [H, tok] /
[DOUT, tok]); the host pre-transposes x and un-transposes the output.
All big matmuls are bf16 (fp32 PSUM accumulation). Gating softmax + the
descending sort (iterative max-extraction) run token-major on DVE; sorted
weights are PE-transposed to [E, tok] and broadcast across partitions with
a one-hot selector matmul. The second matmul accumulates all 8 experts
(and the b2 @ w term) into one PSUM tile, weighting h by the sorted prob
before the matmul.

build_nc(reps=N) emits the compute body N times (weights loaded once);
test.py uses the marginal time between rep counts to measure HW exec time
through the fixed ~8ms axon dispatch overhead.
"""

import sys

if "/opt/trn_rl_repo" not in sys.path:
    sys.path.insert(0, "/opt/trn_rl_repo")

import numpy as np
import ml_dtypes

import concourse.bass as bass
import concourse.mybir as mybir
import concourse.tile as tile
from concourse import bacc
from concourse.masks import make_identity

B, S, D, DOUT = 4, 2048, 1024, 1024
E, H = 8, 256
NCORES = 8
TOK = (B * S) // NCORES  # 1024 tokens per core
DC = D // 128            # 8 contraction chunks over D
HC = H // 128            # 2 chunks over H
OC = DOUT // 128         # 8 chunks over DOUT
TN = TOK // 512          # 2 moving-dim chunks of 512 tokens
TG = TOK // 128          # 8 token groups of 128 (partition tiles)

BF16 = mybir.dt.bfloat16
F32 = mybir.dt.float32

_CACHE = {}
GELU_FUNC = mybir.ActivationFunctionType.Gelu  # sim override hook


def _bcast_inner(ap2d, n):
    """[P, G] (or [P, G, 1]) AP -> [P, G, n] AP with stride-0 innermost."""
    a = [list(d) for d in ap2d.ap]
    if len(a) == 3:
        assert a[2][1] == 1
        a = a[:2]
    return bass.AP(tensor=ap2d.tensor, offset=ap2d.offset, ap=a + [[0, n]])


def build_nc(reps=1, loop_n=None):
    nc = bacc.Bacc("TRN2", target_bir_lowering=False, debug=False,
                   num_devices=NCORES)

    xT_d = nc.dram_tensor("xT", [D, TOK], BF16, kind="ExternalInput")
    Wg_d = nc.dram_tensor("Wg", [D, E], BF16, kind="ExternalInput")
    bg_d = nc.dram_tensor("bg", [1, E], F32, kind="ExternalInput")
    W1_d = nc.dram_tensor("W1", [E, D, H], BF16, kind="ExternalInput")
    b1_d = nc.dram_tensor("b1", [E, HC, 128, 1], F32, kind="ExternalInput")
    W2_d = nc.dram_tensor("W2", [E, H, DOUT], BF16, kind="ExternalInput")
    b2_d = nc.dram_tensor("b2", [E, DOUT], BF16, kind="ExternalInput")
    sel_d = nc.dram_tensor("sel", [E, E, 128], BF16, kind="ExternalInput")
    outT_d = nc.dram_tensor("outT", [DOUT, TOK], F32, kind="ExternalOutput")

    with tile.TileContext(nc) as tc:
        with (
            tc.tile_pool(name="const", bufs=1) as const,
            tc.tile_pool(name="work", bufs=4) as work,
            tc.tile_pool(name="ps_small", bufs=1, space="PSUM") as ps_small,
            tc.tile_pool(name="ps_wb", bufs=2, space="PSUM") as ps_wb,
            tc.tile_pool(name="ps_h", bufs=2, space="PSUM") as ps_h,
            tc.tile_pool(name="ps_out", bufs=3, space="PSUM") as ps_out,
        ):
            # ---- resident inputs ------------------------------------------
            xT_sb = []
            for dc in range(DC):
                t = const.tile([128, TOK], BF16, name=f"xT{dc}")
                nc.sync.dma_start(t, xT_d[dc * 128:(dc + 1) * 128, :])
                xT_sb.append(t)
            Wg_sb = []
            for dc in range(DC):
                t = const.tile([128, E], BF16, name=f"Wg{dc}")
                nc.sync.dma_start(t, Wg_d[dc * 128:(dc + 1) * 128, :])
                Wg_sb.append(t)
            bg_sb = const.tile([1, E], F32, name="bg")
            nc.sync.dma_start(bg_sb, bg_d[:, :])
            ones_sb = const.tile([1, 128], F32, name="ones")
            nc.vector.memset(ones_sb, 1.0)
            # sel[e]: [E, 128] with row e all-ones -> sel[e].T @ wT broadcasts
            # wT row e across 128 partitions (PE needs base_partition 0)
            sel_t = const.tile([E, E, 128], BF16, name="sel")
            nc.sync.dma_start(sel_t, sel_d[:, :, :])
            sel_sb = [sel_t[:, e, :] for e in range(E)]
            ident = const.tile([128, 128], F32, name="ident")
            make_identity(nc, ident)

            W1_sb = [[None] * DC for _ in range(E)]
            b1_sb = [[None] * HC for _ in range(E)]
            for e in range(E):
                for dc in range(DC):
                    t = const.tile([128, H], BF16, name=f"W1_{e}_{dc}")
                    nc.sync.dma_start(t, W1_d[e, dc * 128:(dc + 1) * 128, :])
                    W1_sb[e][dc] = t
                for hc in range(HC):
                    t = const.tile([128, 1], F32, name=f"b1_{e}_{hc}")
                    nc.sync.dma_start(t, b1_d[e, hc, :, :])
                    b1_sb[e][hc] = t
            W2_sb = [[None] * HC for _ in range(E)]
            for e in range(E):
                for hc in range(HC):
                    t = const.tile([128, DOUT], BF16, name=f"W2_{e}_{hc}")
                    nc.sync.dma_start(t, W2_d[e, hc * 128:(hc + 1) * 128, :])
                    W2_sb[e][hc] = t
            b2_sb = const.tile([E, DOUT], BF16, name="b2")
            nc.sync.dma_start(b2_sb, b2_d[:, :])

            if loop_n is not None:
                with tc.For_i(0, loop_n, 1):
                    _emit_body(nc, "lp_", const, work, ps_small, ps_wb, ps_h,
                               ps_out, xT_sb, Wg_sb, bg_sb, ones_sb, sel_sb,
                               ident, W1_sb, b1_sb, W2_sb, b2_sb, outT_d)
            else:
                for rep in range(reps):
                    _emit_body(nc, f"r{rep}_", const, work, ps_small, ps_wb,
                               ps_h, ps_out, xT_sb, Wg_sb, bg_sb, ones_sb,
                               sel_sb, ident, W1_sb, b1_sb, W2_sb, b2_sb,
                               outT_d)

    nc.compile()
    return nc


def _emit_body(nc, R, const, work, ps_small, ps_wb, ps_h, ps_out,
               xT_sb, Wg_sb, bg_sb, ones_sb, sel_sb, ident,
               W1_sb, b1_sb, W2_sb, b2_sb, outT_d):
    # ---- gating: logits token-major [128, tg, e] --------------------------
    L_sb = const.tile([128, TG, E], F32, name=R + "L", tag="L")
    for tg in range(TG):
        psl = ps_small.tile([128, E], F32, name=f"{R}psl{tg}", tag="small")
        for dc in range(DC):
            nc.tensor.matmul(
                psl, xT_sb[dc][:, tg * 128:(tg + 1) * 128], Wg_sb[dc],
                start=(dc == 0), stop=False)
        nc.tensor.matmul(psl, ones_sb, bg_sb, start=False, stop=True)
        nc.scalar.copy(L_sb[:, tg, :], psl)

    # ---- softmax over E (innermost) ---------------------------------------
    mx = const.tile([128, TG], F32, name=R + "mx", tag="mx")
    nc.vector.reduce_max(out=mx, in_=L_sb, axis=mybir.AxisListType.X)
    P8 = const.tile([128, TG, E], F32, name=R + "P8", tag="P8")
    nc.vector.tensor_sub(P8, L_sb, _bcast_inner(mx, E))
    nc.scalar.activation(P8, P8, mybir.ActivationFunctionType.Exp)
    sm = const.tile([128, TG], F32, name=R + "sm", tag="sm")
    nc.vector.reduce_sum(out=sm, in_=P8, axis=mybir.AxisListType.X)
    rs = const.tile([128, TG], F32, name=R + "rs", tag="rs")
    nc.vector.reciprocal(rs, sm)
    nc.vector.tensor_mul(P8, P8, _bcast_inner(rs, E))

    # ---- sort descending: iterative max extraction ------------------------
    ws = const.tile([128, TG, E], F32, name=R + "ws", tag="ws")  # [.., rank]
    eq = const.tile([128, TG, E], F32, name=R + "eq", tag="eq")
    for r in range(E):
        nc.vector.reduce_max(out=ws[:, :, r:r + 1], in_=P8,
                             axis=mybir.AxisListType.X)
        if r < E - 1:
            nc.vector.tensor_tensor(
                eq, P8, _bcast_inner(ws[:, :, r:r + 1], E),
                op=mybir.AluOpType.is_equal)
            # P8 += -2 * eq  (knock out the extracted max)
            nc.vector.scalar_tensor_tensor(
                P8, eq, -2.0, P8,
                op0=mybir.AluOpType.mult, op1=mybir.AluOpType.add)

    # ---- experts ----------------------------------------------------------
    # Emission order keeps PE streaming: expert 0's first matmuls are emitted
    # before the sorted-weight transposes, so the PE fills the DVE sort
    # latency with useful work. Sorted weights go to bf16 hi+lo pairs so the
    # broadcast / b2 matmuls run at bf16 rate (w = hi + lo, exact to 2^-18).
    h_sb = [[const.tile([128, TOK], BF16, name=f"{R}h_{e}_{hc}",
                        tag=f"h_{e}_{hc}")
             for hc in range(HC)] for e in range(E)]
    wT_hi = const.tile([E, TOK], BF16, name=R + "wT_hi", tag="wT_hi")
    wT_lo = const.tile([E, TOK], BF16, name=R + "wT_lo", tag="wT_lo")

    def emit_ph_pair(e, hc):
        # both token halves accumulate in parallel; each lhsT loads once
        phs = [ps_h.tile([128, 512], F32, name=f"{R}ph{e}_{hc}_{t}", tag="ph")
               for t in range(TN)]
        for dc in range(DC):
            w_ap = W1_sb[e][dc][:, hc * 128:(hc + 1) * 128]
            for t in range(TN):
                mm = nc.tensor.matmul(phs[t], w_ap,
                                      xT_sb[dc][:, t * 512:(t + 1) * 512],
                                      start=(dc == 0), stop=(dc == DC - 1))
                if t > 0:
                    mm.ins.ldweights = False
        gts = []
        for t in range(TN):
            gt = work.tile([128, 512], BF16, name=f"{R}gt{e}_{hc}_{t}",
                           tag="gt")
            nc.scalar.activation(gt, phs[t], GELU_FUNC, bias=b1_sb[e][hc])
            gts.append(gt)
        return gts

    def emit_wb(e, tn):
        tsl = slice(tn * 512, (tn + 1) * 512)
        wb = ps_wb.tile([128, 512], F32, name=f"{R}wb{e}_{tn}", tag="wb")
        nc.tensor.matmul(wb, sel_sb[e], wT_hi[:, tsl], start=True, stop=False)
        nc.tensor.matmul(wb, sel_sb[e], wT_lo[:, tsl], start=False, stop=True)
        return wb

    # expert 0: matmuls first (PE busy while DVE sorts)
    gt0 = {}
    for hc in range(HC):
        g = emit_ph_pair(0, hc)
        for tn in range(TN):
            gt0[(tn, hc)] = g[tn]

    # transpose sorted weights to [E(rank), tok] as bf16 hi + lo
    for tg in range(TG):
        gsl = slice(tg * 128, (tg + 1) * 128)
        pst = ps_small.tile([E, 128], F32, name=f"{R}pst{tg}", tag="small")
        nc.tensor.transpose(pst, ws[:, tg, :], ident)
        nc.scalar.copy(wT_hi[:, gsl], pst)
        # lo = pst - hi
        nc.vector.scalar_tensor_tensor(wT_lo[:, gsl], wT_hi[:, gsl], -1.0,
                                       pst, op0=mybir.AluOpType.mult,
                                       op1=mybir.AluOpType.add)

    for tn in range(TN):
        wb = emit_wb(0, tn)
        for hc in range(HC):
            tsl = slice(tn * 512, (tn + 1) * 512)
            nc.vector.tensor_mul(h_sb[0][hc][:, tsl], gt0[(tn, hc)], wb)

    for e in range(1, E):
        wbs = [emit_wb(e, tn) for tn in range(TN)]
        for hc in range(HC):
            gts = emit_ph_pair(e, hc)
            for tn in range(TN):
                tsl = slice(tn * 512, (tn + 1) * 512)
                nc.vector.tensor_mul(h_sb[e][hc][:, tsl], gts[tn], wbs[tn])

    for oc in range(OC):
        pos = [ps_out.tile([128, 512], F32, name=f"{R}po{oc}_{t}", tag="po")
               for t in range(TN)]
        b2_ap = b2_sb[:, oc * 128:(oc + 1) * 128]
        for t in range(TN):
            mm = nc.tensor.matmul(pos[t], b2_ap,
                                  wT_hi[:, t * 512:(t + 1) * 512],
                                  start=True, stop=False)
            if t > 0:
                mm.ins.ldweights = False
        for e in range(E):
            for hc in range(HC):
                w_ap = W2_sb[e][hc][:, oc * 128:(oc + 1) * 128]
                for t in range(TN):
                    mm = nc.tensor.matmul(
                        pos[t], w_ap, h_sb[e][hc][:, t * 512:(t + 1) * 512],
                        start=False, stop=(e == E - 1 and hc == HC - 1))
                    if t > 0:
                        mm.ins.ldweights = False
        for t in range(TN):
            ot = work.tile([128, 512], F32, name=f"{R}ot{oc}_{t}", tag="ot")
            nc.scalar.copy(ot, pos[t])
            nc.sync.dma_start(
                outT_d[oc * 128:(oc + 1) * 128, t * 512:(t + 1) * 512], ot)


def _prep_in_maps(x, Wg, bg, W1, b1, W2, b2):
    x = np.asarray(x, dtype=np.float32).reshape(B * S, D)
    Wg_bf = np.asarray(Wg, dtype=np.float32).astype(ml_dtypes.bfloat16)
    bg_f = np.asarray(bg, dtype=np.float32).reshape(1, E)
    W1_bf = np.asarray(W1, dtype=np.float32).astype(ml_dtypes.bfloat16)
    b1_f = np.ascontiguousarray(
        np.asarray(b1, dtype=np.float32).reshape(E, HC, 128, 1))
    W2_bf = np.asarray(W2, dtype=np.float32).astype(ml_dtypes.bfloat16)
    b2_f = np.asarray(b2, dtype=np.float32).astype(ml_dtypes.bfloat16)
    sel_np = np.zeros((E, E, 128), dtype=np.float32).astype(ml_dtypes.bfloat16)
    for e in range(E):
        sel_np[e, e, :] = 1.0
    in_maps = []
    for c in range(NCORES):
        xc = x[c * TOK:(c + 1) * TOK]                      # [TOK, D]
        xT = np.ascontiguousarray(xc.T).astype(ml_dtypes.bfloat16)
        in_maps.append({
            "xT": xT, "Wg": Wg_bf, "bg": bg_f, "W1": W1_bf,
            "b1": b1_f, "W2": W2_bf, "b2": b2_f, "sel": sel_np,
        })
    return in_maps


def kernel(x, Wg, bg, W1, b1, W2, b2):
    from concourse.bass_utils import run_bass_kernel_spmd

    if "nc" not in _CACHE:
        _CACHE["nc"] = build_nc()
    nc = _CACHE["nc"]
    in_maps = _prep_in_maps(x, Wg, bg, W1, b1, W2, b2)
    res = run_bass_kernel_spmd(nc, in_maps, core_ids=list(range(NCORES)))
    out = np.empty((B * S, DOUT), dtype=np.float32)
    for c in range(NCORES):
        out[c * TOK:(c + 1) * TOK] = res.results[c]["outT"].T
    return out.reshape(B, S, DOUT)


# revision 17
# speedup vs baseline: 1.3260x; 1.3260x over previous
"""MoE layer (dense routing, E=8 experts, top_k=E) Trainium2 Bass kernel.

Problem (hardcoded): x [4, 2048, 1024] f32, Wg [1024, 8], bg [8],
W1 [8, 1024, 256], b1 [8, 256], W2 [8, 256, 1024], b2 [8, 1024].

reference:
    logits = x @ Wg + bg ; probs = softmax(logits)
    sorted_probs = sort(probs, descending)          # top_k with k=E
    h_e = gelu(x @ W1[e] + b1[e])                   # all experts, all tokens
    out = sum_e (h_e @ W2[e] + b2[e]) * sorted_probs[..., e, None]

Sharding: data-parallel over the 8192 tokens -> 1024 tokens/core, 8 cores,
weights replicated, no collectives.

Per-core layout: activations are feature-major ([D, tok] / [H, tok] /
[DOUT, tok]); the host pre-transposes x and un-transposes the output.
All big matmuls are bf16 (fp32 PSUM accumulation). Gating softmax + the
descending sort (iterative max-extraction) run token-major on DVE; sorted
weights are PE-transposed to [E, tok] and broadcast across partitions with
a one-hot selector matmul. The second matmul accumulates all 8 experts
(and the b2 @ w term) into one PSUM tile, weighting h by the sorted prob
before the matmul.

build_nc(reps=N) emits the compute body N times (weights loaded once);
test.py uses the marginal time between rep counts to measure HW exec time
through the fixed ~8ms axon dispatch overhead.
"""

import sys

if "/opt/trn_rl_repo" not in sys.path:
    sys.path.insert(0, "/opt/trn_rl_repo")

import numpy as np
import ml_dtypes

import concourse.bass as bass
import concourse.mybir as mybir
import concourse.tile as tile
from concourse import bacc
from concourse.masks import make_identity

B, S, D, DOUT = 4, 2048, 1024, 1024
E, H = 8, 256
NCORES = 8
TOK = (B * S) // NCORES  # 1024 tokens per core
DC = D // 128            # 8 contraction chunks over D
HC = H // 128            # 2 chunks over H
OC = DOUT // 128         # 8 chunks over DOUT
TN = TOK // 512          # 2 moving-dim chunks of 512 tokens
TG = TOK // 128          # 8 token groups of 128 (partition tiles)

BF16 = mybir.dt.bfloat16
F32 = mybir.dt.float32

_CACHE = {}
GELU_FUNC = mybir.ActivationFunctionType.Gelu  # sim override hook


def _bcast_inner(ap2d, n):
    """[P, G] (or [P, G, 1]) AP -> [P, G, n] AP with stride-0 innermost."""
    a = [list(d) for d in ap2d.ap]
    if len(a) == 3:
        assert a[2][1] == 1
        a = a[:2]
    return bass.AP(tensor=ap2d.tensor, offset=ap2d.offset, ap=a + [[0, n]])


def build_nc(reps=1, loop_n=None):
    nc = bacc.Bacc("TRN2", target_bir_lowering=False, debug=False,
                   num_devices=NCORES)

    xT_d = nc.dram_tensor("xT", [D, TOK], BF16, kind="ExternalInput")
    Wg_d = nc.dram_tensor("Wg", [D, E], BF16, kind="ExternalInput")
    bg_d = nc.dram_tensor("bg", [1, E], F32, kind="ExternalInput")
    W1_d = nc.dram_tensor("W1", [E, D, H], BF16, kind="ExternalInput")
    b1_d = nc.dram_tensor("b1", [E, HC, 128, 1], F32, kind="ExternalInput")
    W2_d = nc.dram_tensor("W2", [E, H, DOUT], BF16, kind="ExternalInput")
    b2_d = nc.dram_tensor("b2", [E, DOUT], BF16, kind="ExternalInput")
    sel_d = nc.dram_tensor("sel", [E, E, 128], BF16, kind="ExternalInput")
    outT_d = nc.dram_tensor("outT", [DOUT, TOK], F32, kind="ExternalOutput")

    with tile.TileContext(nc) as tc:
        with (
            tc.tile_pool(name="const", bufs=1) as const,
            tc.tile_pool(name="work", bufs=4) as work,
            tc.tile_pool(name="ps_small", bufs=2, space="PSUM") as ps_small,
            tc.tile_pool(name="ps_wb", bufs=2, space="PSUM") as ps_wb,
            tc.tile_pool(name="ps_h", bufs=2, space="PSUM") as ps_h,
            tc.tile_pool(name="ps_out", bufs=2, space="PSUM") as ps_out,
        ):
            # ---- resident inputs ------------------------------------------
            xT_sb = []
            for dc in range(DC):
                t = const.tile([128, TOK], BF16, name=f"xT{dc}")
                nc.sync.dma_start(t, xT_d[dc * 128:(dc + 1) * 128, :])
                xT_sb.append(t)
            Wg_sb = []
            for dc in range(DC):
                t = const.tile([128, E], BF16, name=f"Wg{dc}")
                nc.sync.dma_start(t, Wg_d[dc * 128:(dc + 1) * 128, :])
                Wg_sb.append(t)
            bg_sb = const.tile([1, E], F32, name="bg")
            nc.sync.dma_start(bg_sb, bg_d[:, :])
            ones_sb = const.tile([1, 128], F32, name="ones")
            nc.vector.memset(ones_sb, 1.0)
            # sel[e]: [E, 128] with row e all-ones -> sel[e].T @ wT broadcasts
            # wT row e across 128 partitions (PE needs base_partition 0)
            sel_t = const.tile([E, E, 128], BF16, name="sel")
            nc.sync.dma_start(sel_t, sel_d[:, :, :])
            sel_sb = [sel_t[:, e, :] for e in range(E)]
            ident = const.tile([128, 128], F32, name="ident")
            make_identity(nc, ident)

            W1_sb = [[None] * DC for _ in range(E)]
            b1_sb = [[None] * HC for _ in range(E)]
            for e in range(E):
                for dc in range(DC):
                    t = const.tile([128, H], BF16, name=f"W1_{e}_{dc}")
                    nc.sync.dma_start(t, W1_d[e, dc * 128:(dc + 1) * 128, :])
                    W1_sb[e][dc] = t
                for hc in range(HC):
                    t = const.tile([128, 1], F32, name=f"b1_{e}_{hc}")
                    nc.sync.dma_start(t, b1_d[e, hc, :, :])
                    b1_sb[e][hc] = t
            W2_sb = [[None] * HC for _ in range(E)]
            for e in range(E):
                for hc in range(HC):
                    t = const.tile([128, DOUT], BF16, name=f"W2_{e}_{hc}")
                    nc.sync.dma_start(t, W2_d[e, hc * 128:(hc + 1) * 128, :])
                    W2_sb[e][hc] = t
            b2_sb = const.tile([E, DOUT], BF16, name="b2")
            nc.sync.dma_start(b2_sb, b2_d[:, :])

            if loop_n is not None:
                with tc.For_i(0, loop_n, 1):
                    _emit_body(nc, "lp_", const, work, ps_small, ps_wb, ps_h,
                               ps_out, xT_sb, Wg_sb, bg_sb, ones_sb, sel_sb,
                               ident, W1_sb, b1_sb, W2_sb, b2_sb, outT_d)
            else:
                for rep in range(reps):
                    _emit_body(nc, f"r{rep}_", const, work, ps_small, ps_wb,
                               ps_h, ps_out, xT_sb, Wg_sb, bg_sb, ones_sb,
                               sel_sb, ident, W1_sb, b1_sb, W2_sb, b2_sb,
                               outT_d)

    nc.compile()
    return nc


def _emit_body(nc, R, const, work, ps_small, ps_wb, ps_h, ps_out,
               xT_sb, Wg_sb, bg_sb, ones_sb, sel_sb, ident,
               W1_sb, b1_sb, W2_sb, b2_sb, outT_d):
    # ---- gating: logits token-major [128, tg, e] --------------------------
    L_sb = const.tile([128, TG, E], F32, name=R + "L", tag="L")
    for tg in range(TG):
        psl = ps_small.tile([128, E], F32, name=f"{R}psl{tg}", tag="small")
        for dc in range(DC):
            nc.tensor.matmul(
                psl, xT_sb[dc][:, tg * 128:(tg + 1) * 128], Wg_sb[dc],
                start=(dc == 0), stop=False)
        nc.tensor.matmul(psl, ones_sb, bg_sb, start=False, stop=True)
        nc.scalar.copy(L_sb[:, tg, :], psl)

    # ---- softmax over E (innermost) ---------------------------------------
    mx = const.tile([128, TG], F32, name=R + "mx", tag="mx")
    nc.vector.reduce_max(out=mx, in_=L_sb, axis=mybir.AxisListType.X)
    P8 = const.tile([128, TG, E], F32, name=R + "P8", tag="P8")
    nc.vector.tensor_sub(P8, L_sb, _bcast_inner(mx, E))
    nc.scalar.activation(P8, P8, mybir.ActivationFunctionType.Exp)
    sm = const.tile([128, TG], F32, name=R + "sm", tag="sm")
    nc.vector.reduce_sum(out=sm, in_=P8, axis=mybir.AxisListType.X)
    rs = const.tile([128, TG], F32, name=R + "rs", tag="rs")
    nc.vector.reciprocal(rs, sm)
    nc.vector.tensor_mul(P8, P8, _bcast_inner(rs, E))

    # ---- sort descending: iterative max extraction ------------------------
    ws = const.tile([128, TG, E], F32, name=R + "ws", tag="ws")  # [.., rank]
    eq = const.tile([128, TG, E], F32, name=R + "eq", tag="eq")
    for r in range(E):
        nc.vector.reduce_max(out=ws[:, :, r:r + 1], in_=P8,
                             axis=mybir.AxisListType.X)
        if r < E - 1:
            nc.vector.tensor_tensor(
                eq, P8, _bcast_inner(ws[:, :, r:r + 1], E),
                op=mybir.AluOpType.is_equal)
            # P8 += -2 * eq  (knock out the extracted max)
            nc.vector.scalar_tensor_tensor(
                P8, eq, -2.0, P8,
                op0=mybir.AluOpType.mult, op1=mybir.AluOpType.add)

    # ---- experts ----------------------------------------------------------
    # Emission order keeps PE streaming: expert 0's first matmuls are emitted
    # before the sorted-weight transposes, so the PE fills the DVE sort
    # latency with useful work. Sorted weights go to bf16 hi+lo pairs so the
    # broadcast / b2 matmuls run at bf16 rate (w = hi + lo, exact to 2^-18).
    h_sb = [[const.tile([128, TOK], BF16, name=f"{R}h_{e}_{hc}",
                        tag=f"h_{e}_{hc}")
             for hc in range(HC)] for e in range(E)]
    wT_hi = const.tile([E, TOK], BF16, name=R + "wT_hi", tag="wT_hi")
    wT_lo = const.tile([E, TOK], BF16, name=R + "wT_lo", tag="wT_lo")

    def emit_ph_pair(e, hc):
        # both token halves accumulate in parallel; each lhsT loads once
        phs = [ps_h.tile([128, 512], F32, name=f"{R}ph{e}_{hc}_{t}", tag="ph")
               for t in range(TN)]
        for dc in range(DC):
            w_ap = W1_sb[e][dc][:, hc * 128:(hc + 1) * 128]
            for t in range(TN):
                mm = nc.tensor.matmul(phs[t], w_ap,
                                      xT_sb[dc][:, t * 512:(t + 1) * 512],
                                      start=(dc == 0), stop=(dc == DC - 1))
                if t > 0:
                    mm.ins.ldweights = False
        gts = []
        for t in range(TN):
            gt = work.tile([128, 512], BF16, name=f"{R}gt{e}_{hc}_{t}",
                           tag="gt")
            nc.scalar.activation(gt, phs[t], GELU_FUNC, bias=b1_sb[e][hc])
            gts.append(gt)
        return gts

    def emit_wb(e, tn):
        tsl = slice(tn * 512, (tn + 1) * 512)
        wb = ps_wb.tile([128, 512], F32, name=f"{R}wb{e}_{tn}", tag="wb")
        nc.tensor.matmul(wb, sel_sb[e], wT_hi[:, tsl], start=True, stop=False)
        nc.tensor.matmul(wb, sel_sb[e], wT_lo[:, tsl], start=False, stop=True)
        return wb

    # expert 0: matmuls first (PE busy while DVE sorts)
    gt0 = {}
    for hc in range(HC):
        g = emit_ph_pair(0, hc)
        for tn in range(TN):
            gt0[(tn, hc)] = g[tn]

    # transpose sorted weights to [E(rank), tok] as bf16 hi + lo
    for tg in range(TG):
        gsl = slice(tg * 128, (tg + 1) * 128)
        pst = ps_small.tile([E, 128], F32, name=f"{R}pst{tg}", tag="small")
        nc.tensor.transpose(pst, ws[:, tg, :], ident)
        nc.scalar.copy(wT_hi[:, gsl], pst)
        # lo = pst - hi
        nc.vector.scalar_tensor_tensor(wT_lo[:, gsl], wT_hi[:, gsl], -1.0,
                                       pst, op0=mybir.AluOpType.mult,
                                       op1=mybir.AluOpType.add)

    for tn in range(TN):
        wb = emit_wb(0, tn)
        for hc in range(HC):
            tsl = slice(tn * 512, (tn + 1) * 512)
            nc.vector.tensor_mul(h_sb[0][hc][:, tsl], gt0[(tn, hc)], wb)

    for e in range(1, E):
        wbs = [emit_wb(e, tn) for tn in range(TN)]
        for hc in range(HC):
            gts = emit_ph_pair(e, hc)
            for tn in range(TN):
                tsl = slice(tn * 512, (tn + 1) * 512)
                nc.vector.tensor_mul(h_sb[e][hc][:, tsl], gts[tn], wbs[tn])

    for oc in range(OC):
        pos = [ps_out.tile([128, 512], F32, name=f"{R}po{oc}_{t}", tag="po")
               for t in range(TN)]
        b2_ap = b2_sb[:, oc * 128:(oc + 1) * 128]
        for t in range(TN):
            mm = nc.tensor.matmul(pos[t], b2_ap,
                                  wT_hi[:, t * 512:(t + 1) * 512],
                                  start=True, stop=False)
            if t > 0:
                mm.ins.ldweights = False
        for e in range(E):
            for hc in range(HC):
                w_ap = W2_sb[e][hc][:, oc * 128:(oc + 1) * 128]
                for t in range(TN):
                    mm = nc.tensor.matmul(
                        pos[t], w_ap, h_sb[e][hc][:, t * 512:(t + 1) * 512],
                        start=False, stop=(e == E - 1 and hc == HC - 1))
                    if t > 0:
                        mm.ins.ldweights = False
        for t in range(TN):
            ot = work.tile([128, 512], F32, name=f"{R}ot{oc}_{t}", tag="ot")
            nc.scalar.copy(ot, pos[t])
            nc.sync.dma_start(
                outT_d[oc * 128:(oc + 1) * 128, t * 512:(t + 1) * 512], ot)


def _prep_in_maps(x, Wg, bg, W1, b1, W2, b2):
    x = np.asarray(x, dtype=np.float32).reshape(B * S, D)
    Wg_bf = np.asarray(Wg, dtype=np.float32).astype(ml_dtypes.bfloat16)
    bg_f = np.asarray(bg, dtype=np.float32).reshape(1, E)
    W1_bf = np.asarray(W1, dtype=np.float32).astype(ml_dtypes.bfloat16)
    b1_f = np.ascontiguousarray(
        np.asarray(b1, dtype=np.float32).reshape(E, HC, 128, 1))
    W2_bf = np.asarray(W2, dtype=np.float32).astype(ml_dtypes.bfloat16)
    b2_f = np.asarray(b2, dtype=np.float32).astype(ml_dtypes.bfloat16)
    sel_np = np.zeros((E, E, 128), dtype=np.float32).astype(ml_dtypes.bfloat16)
    for e in range(E):
        sel_np[e, e, :] = 1.0
    in_maps = []
    for c in range(NCORES):
        xc = x[c * TOK:(c + 1) * TOK]                      # [TOK, D]
        xT = np.ascontiguousarray(xc.T).astype(ml_dtypes.bfloat16)
        in_maps.append({
            "xT": xT, "Wg": Wg_bf, "bg": bg_f, "W1": W1_bf,
            "b1": b1_f, "W2": W2_bf, "b2": b2_f, "sel": sel_np,
        })
    return in_maps


def kernel(x, Wg, bg, W1, b1, W2, b2):
    from concourse.bass_utils import run_bass_kernel_spmd

    if "nc" not in _CACHE:
        _CACHE["nc"] = build_nc()
    nc = _CACHE["nc"]
    in_maps = _prep_in_maps(x, Wg, bg, W1, b1, W2, b2)
    res = run_bass_kernel_spmd(nc, in_maps, core_ids=list(range(NCORES)))
    out = np.empty((B * S, DOUT), dtype=np.float32)
    for c in range(NCORES):
        out[c * TOK:(c + 1) * TOK] = res.results[c]["outT"].T
    return out.reshape(B, S, DOUT)
